# revision 4
# baseline (speedup 1.0000x reference)
"""DD-RoPE kernel for 8x TRN2 NeuronCores — pipelined "t-on-partitions" v2.

Reference computation (B=4, T=4096, D=2048, P=256):
    deltas = einsum('btd,pd->btp', x, W) + b     # (B, T, P)
    angles = cumsum(deltas, axis=1)
    out = concat([x1*cos(a) - x2*sin(a), x2*cos(a) + x1*sin(a), x[..., 512:]], -1)

Sharding: 8 shards = 4 batches x 2 T-halves (2048 steps each), data-parallel.
The cumsum is split into independent 128-step blocks via host-computed fp64
block bases (exact cumulative angle at each 128-step boundary) injected on
device through a rank-4 affine matmul, so no cross-core communication and
bounded within-block drift.

v2 vs v1: the TRN2 PE p-states (full 2.4 GHz only after 3us of continuous
execution; idle resets the ramp) make PE bubbles doubly expensive, and the
16us DMA prologue + coarse 4-block groups of v1 serialized the engines.
v2 therefore:
  - processes PAIRS of 128-step blocks (tiles [128t, 512]) with a 1-pair
    software skew: PE streams delta matmuls back-to-back while the previous
    pair's cumsum/trig/rotate runs on Scalar/Vector,
  - fine-grained prologue: first w/xt quarter-DMAs are high-priority so the
    first matmul starts as soon as ~384KB has landed (PE ramps while DMA
    streams),
  - issues x12/out/const DMAs from the idle GpSimd queue (25ns sequencer
    cost vs 565ns on SP) so SP only sequences the xt/w stream,
  - warms the Sin activation table during the prologue (dummy Sin), keeping
    the 1.3us ACT_TABLE_LOAD off the critical path,
  - tapers the tail: the last two blocks run as single-block (256-wide)
    chains so the post-matmul drain is short.

Per-pair engine budget (cost model): PE 32 MM x 109ns + 4 angle MM = 3.9us,
Scalar 4 acts ~2.1us, DVE 8 ops ~3.2us, DMA 1.5MiB ~4.4us.
"""

import sys

if "/opt/trn_rl_repo" not in sys.path:
    sys.path.insert(0, "/opt/trn_rl_repo")

from contextlib import ExitStack

import numpy as np

import concourse.bacc as bacc
import concourse.bass as bass
import concourse.mybir as mybir
import concourse.tile as tile
from concourse.bass_utils import run_bass_kernel_spmd

F32 = mybir.dt.float32
F16 = mybir.dt.float16
ADD = mybir.AluOpType.add
SUB = mybir.AluOpType.subtract
MULT = mybir.AluOpType.mult
MAX = mybir.AluOpType.max
IDENT = mybir.ActivationFunctionType.Identity
SIN = mybir.ActivationFunctionType.Sin

D = 2048          # input feature dim (contraction)
P = 256           # delta-pairs dim
ROT = 2 * P       # rotated columns (512)
TL = 2048         # time steps per shard
BK = 128          # cumsum block (base injection granularity)
NBK = TL // BK    # blocks per shard (16)
KC = D // 128     # contraction chunks (16)
NPAIR = NBK // 2  # row-pairs in the dram layouts (8)
N_CORES = 8

# pipeline items: (block offset, blocks in item) — tapered single-block tail
ITEMS = [(0, 2), (2, 2), (4, 2), (6, 2), (8, 2), (10, 2), (12, 2),
         (14, 1), (15, 1)]

MAGIC = 12582912.0          # 1.5 * 2**23: fp32 round-to-int magic constant
SCALE_2PI = 6.28310         # slightly < 2*pi so Sin args stay inside [-pi, pi]
HALF_PI = 1.5707964


def build_program() -> bass.Bass:
    nc = bacc.Bacc("TRN2", target_bir_lowering=False, debug=False)

    # x^T tiles: [r*128 + dp, (bkl*KC + dc)*128 + tl] = xs[(2r+bkl)*128+tl,
    #                                                      dc*128 + dp]
    xt = nc.dram_tensor("xt", [NPAIR * 128, 2 * KC * 128], F16,
                        kind="ExternalInput").ap()
    # W, d-chunks along free: [128 d-part, dc*P + p] fp16
    w = nc.dram_tensor("w", [128, KC * P], F16, kind="ExternalInput").ap()
    # upper-triangular ones (u[t, t'] = 1 iff t <= t')
    u = nc.dram_tensor("u", [128, 128], F16, kind="ExternalInput").ap()
    # affine stationary: rows [ones, ones, ramp(1..128), ramp]
    afs = nc.dram_tensor("afs", [4, 128], F16, kind="ExternalInput").ap()
    # affine moving: rows [base_hi[bk,p], base_lo, b_hi, b_lo], bk-major
    afm = nc.dram_tensor("afm", [4, NBK * P], F16, kind="ExternalInput").ap()
    # rotation operands: [r*128 + tl, half*512 + bkl*256 + p]
    x12 = nc.dram_tensor("x12", [NPAIR * 128, 2 * ROT], F16,
                         kind="ExternalInput").ap()
    # rotated output, same indexing as x12
    outT = nc.dram_tensor("outT", [NPAIR * 128, 2 * ROT], F16,
                          kind="ExternalOutput").ap()

    with tile.TileContext(nc) as tc, ExitStack() as ctx:
        const_pool = ctx.enter_context(tc.tile_pool(name="const", bufs=1))
        w_pool = ctx.enter_context(tc.tile_pool(name="w", bufs=1))
        xt_pool = ctx.enter_context(tc.tile_pool(name="xt", bufs=3))
        x12_pool = ctx.enter_context(tc.tile_pool(name="x12", bufs=3))
        dp_pool = ctx.enter_context(
            tc.tile_pool(name="dp_psum", bufs=3, space="PSUM"))
        ang_pool = ctx.enter_context(
            tc.tile_pool(name="ang_psum", bufs=2, space="PSUM"))
        d16_pool = ctx.enter_context(tc.tile_pool(name="d16", bufs=2))
        a32_pool = ctx.enter_context(tc.tile_pool(name="a32", bufs=2))
        trig_pool = ctx.enter_context(tc.tile_pool(name="trig", bufs=2))
        rot_pool = ctx.enter_context(tc.tile_pool(name="rot", bufs=2))
        out_pool = ctx.enter_context(tc.tile_pool(name="out", bufs=2))

        # --- prologue: critical first transfers + act-table warmup -------
        w_sb = w_pool.tile([128, KC * P], F16, tag="w")
        # first quarter of w (dc 0..3) gates the very first matmul
        nc.sync.dma_start(w_sb[:, 0:4 * P], w[:, 0:4 * P])

        u_sb = const_pool.tile([128, 128], F16, tag="u")
        afs_sb = const_pool.tile([4, 128], F16, tag="afs")
        afm_sb = const_pool.tile([4, NBK * P], F16, tag="afm")
        nc.gpsimd.dma_start(u_sb[:], u[:])
        nc.gpsimd.dma_start(afs_sb[:], afs[:])
        nc.gpsimd.dma_start(afm_sb[:], afm[:])
        magic_sb = const_pool.tile([128, 1], F32, tag="magic")
        nc.gpsimd.memset(magic_sb[:], MAGIC)
        hpi_sb = const_pool.tile([128, 1], F32, tag="hpi")
        nc.gpsimd.memset(hpi_sb[:], HALF_PI)
        # dummy Sin pulls the 1.3us ACT_TABLE_LOAD into the prologue
        warm_sb = const_pool.tile([128, 1], F16, tag="warm")
        nc.gpsimd.memset(warm_sb[:], 0.0)
        warm2_sb = const_pool.tile([128, 1], F16, tag="warm2")
        nc.scalar.activation(warm2_sb[:], warm_sb[:], SIN)

        def wid_of(it):
            return ITEMS[it][1] * P

        def issue_in_dmas(it):
            bo, nb = ITEMS[it]
            r, lo = bo // 2, bo % 2
            rows = slice(r * 128, (r + 1) * 128)
            xtg = xt_pool.tile([128, nb * KC * 128], F16, tag="xt")
            xsl = slice(lo * KC * 128, (lo + nb) * KC * 128)
            if it == 0:
                # split so (bk0, dc0..3) lands first and PE starts early
                nc.sync.dma_start(xtg[:, 0:4 * 128], xt[rows, 0:4 * 128])
                nc.sync.dma_start(xtg[:, 4 * 128:KC * 128],
                                  xt[rows, 4 * 128:KC * 128])
                nc.sync.dma_start(xtg[:, KC * 128:2 * KC * 128],
                                  xt[rows, KC * 128:2 * KC * 128])
            else:
                nc.sync.dma_start(xtg[:], xt[rows, xsl])
            x12t = x12_pool.tile([128, 2 * wid_of(it)], F16, tag="x12")
            if nb == 2:
                nc.gpsimd.dma_start(x12t[:], x12[rows, :])
            else:
                # single block: two 256-wide column slices (x1 and x2 halves)
                nc.gpsimd.dma_start(
                    x12t[:, 0:P], x12[rows, lo * P:(lo + 1) * P])
                nc.gpsimd.dma_start(
                    x12t[:, P:2 * P],
                    x12[rows, ROT + lo * P:ROT + (lo + 1) * P])
            return xtg, x12t

        def stage_deltas(it, xtg):
            bo, nb = ITEMS[it]
            wid = nb * P
            dp = dp_pool.tile([128, wid], F32, tag="dp")
            for bkl in range(nb):
                sl = slice(bkl * P, (bkl + 1) * P)
                for dc in range(KC):
                    nc.tensor.matmul(
                        dp[:, sl],
                        xtg[:, (bkl * KC + dc) * 128:(bkl * KC + dc + 1) * 128],
                        w_sb[:, dc * P:(dc + 1) * P],
                        start=(dc == 0), stop=(dc == KC - 1))
            d16 = d16_pool.tile([128, wid], F16, tag="d16")
            nc.scalar.activation(d16[:], dp[:], IDENT)
            return d16

        def stage_back(it, d16, x12t):
            """Angle matmuls + trig + rotation + out DMA for item `it`."""
            bo, nb = ITEMS[it]
            wid = nb * P
            r, lo = bo // 2, bo % 2
            rows = slice(r * 128, (r + 1) * 128)
            ang = ang_pool.tile([128, wid], F32, tag="ang")
            for bkl in range(nb):
                bk = bo + bkl
                sl = slice(bkl * P, (bkl + 1) * P)
                nc.tensor.matmul(ang[:, sl], u_sb[:], d16[:, sl],
                                 start=True, stop=False)
                nc.tensor.matmul(ang[:, sl], afs_sb[:],
                                 afm_sb[:, bk * P:(bk + 1) * P],
                                 start=False, stop=True)

            # range reduction (turns): rs = y - round(y) in [-0.5, 0.5]
            a_s = a32_pool.tile([128, wid], F32, tag="a_s")
            nc.scalar.activation(a_s[:], ang[:], IDENT,
                                 bias=magic_sb[:], scale=-1.0)
            rs = trig_pool.tile([128, wid], F16, tag="rs")
            nc.vector.scalar_tensor_tensor(rs[:], a_s[:], MAGIC, ang[:],
                                           op0=SUB, op1=ADD)
            sn = trig_pool.tile([128, wid], F16, tag="sn")
            nc.scalar.activation(sn[:], rs[:], SIN, scale=SCALE_2PI)
            # cos(2pi*y) = sin(pi/2 - 2pi*|rs|), same reduction
            ra = trig_pool.tile([128, wid], F16, tag="ra")
            nc.vector.scalar_tensor_tensor(ra[:], rs[:], -1.0, rs[:],
                                           op0=MULT, op1=MAX)
            cs = trig_pool.tile([128, wid], F16, tag="cs")
            nc.scalar.activation(cs[:], ra[:], SIN,
                                 scale=-SCALE_2PI, bias=hpi_sb[:])

            # rotation, all-fp16 on DVE
            x1 = x12t[:, 0:wid]
            x2 = x12t[:, wid:2 * wid]
            o = out_pool.tile([128, 2 * wid], F16, tag="o")
            t1 = rot_pool.tile([128, wid], F16, tag="t1")
            nc.vector.tensor_mul(t1[:], x1, cs[:])
            t2 = rot_pool.tile([128, wid], F16, tag="t2")
            nc.vector.tensor_mul(t2[:], x2, sn[:])
            nc.vector.tensor_sub(o[:, 0:wid], t1[:], t2[:])
            t3 = rot_pool.tile([128, wid], F16, tag="t3")
            nc.vector.tensor_mul(t3[:], x2, cs[:])
            t4 = rot_pool.tile([128, wid], F16, tag="t4")
            nc.vector.tensor_mul(t4[:], x1, sn[:])
            nc.vector.tensor_add(o[:, wid:2 * wid], t3[:], t4[:])

            if nb == 2:
                nc.gpsimd.dma_start(outT[rows, :], o[:])
            else:
                nc.gpsimd.dma_start(
                    outT[rows, lo * P:(lo + 1) * P], o[:, 0:P])
                nc.gpsimd.dma_start(
                    outT[rows, ROT + lo * P:ROT + (lo + 1) * P],
                    o[:, P:2 * P])

        # remaining w quarters right after the first xt DMAs in SP order
        def issue_w_rest():
            for q in range(1, 4):
                nc.sync.dma_start(w_sb[:, q * 4 * P:(q + 1) * 4 * P],
                                  w[:, q * 4 * P:(q + 1) * 4 * P])

        pend = None  # (it, d16, x12t) awaiting its back stage
        for it in range(len(ITEMS)):
            xtg, x12t = issue_in_dmas(it)
            if it == 0:
                issue_w_rest()
            d16 = stage_deltas(it, xtg)
            if pend is not None:
                stage_back(*pend)
            pend = (it, d16, x12t)
        stage_back(*pend)

    nc.compile()
    return nc


_NC_CACHE: dict = {}


def _get_nc():
    if "nc" not in _NC_CACHE:
        _NC_CACHE["nc"] = build_program()
    return _NC_CACHE["nc"]


def prepare_weights(W: np.ndarray, b: np.ndarray):
    inv2pi = 1.0 / (2.0 * np.pi)
    Wt = W.astype(np.float64).T * inv2pi                       # [D, P]
    wh = Wt.astype(np.float16)
    bt = b.astype(np.float64) * inv2pi                         # [P]
    bh = bt.astype(np.float16)
    bl = (bt - bh.astype(np.float64)).astype(np.float16)
    # [D, P] -> [128, KC*P] with d-chunks along the free dim
    w_in = np.ascontiguousarray(
        wh.reshape(KC, 128, P).transpose(1, 0, 2).reshape(128, KC * P))
    # Bases must come from the FULL-precision weights so each 128-step block
    # restarts at the reference-exact angle: the device's fp16-W error then
    # only drifts within one block instead of accumulating across the shard.
    return w_in, bh, bl, Wt, bt


def make_in_maps(x: np.ndarray, W: np.ndarray, b: np.ndarray):
    B, T, _ = x.shape
    w_in, bh, bl, w_eff, b_eff = prepare_weights(W, b)

    u_in = np.triu(np.ones((128, 128), np.float16))
    afs_in = np.stack([
        np.ones(128, np.float16), np.ones(128, np.float16),
        np.arange(1, 129, dtype=np.float16),
        np.arange(1, 129, dtype=np.float16)])

    # fp64 cumulative angle at every 128-step boundary, per batch (turns)
    nblk = T // BK                                              # 32
    xblk = x.reshape(B, nblk, BK, D).sum(axis=2, dtype=np.float64)
    dblk = xblk @ w_eff + BK * b_eff                            # [B, 32, P]
    bases = np.zeros((B, nblk, P))
    np.cumsum(dblk[:, :-1], axis=1, out=bases[:, 1:])           # exclusive

    in_maps = []
    for c in range(N_CORES):
        bb, hh = c // 2, c % 2
        xs = x[bb, hh * TL:(hh + 1) * TL, :].astype(np.float16)  # [TL, D]
        # xt: [r*128 + dp, (bkl*KC + dc)*128 + tl]
        xt_in = np.ascontiguousarray(
            xs.reshape(NPAIR, 2, BK, KC, 128).transpose(0, 4, 1, 3, 2)
            .reshape(NPAIR * 128, 2 * KC * 128))
        # x12: [r*128 + tl, half*ROT + bkl*P + p]
        x12_in = np.ascontiguousarray(
            xs[:, :ROT].reshape(NPAIR, 2, BK, 2, P).transpose(0, 2, 3, 1, 4)
            .reshape(NPAIR * 128, 2 * ROT))
        bs = bases[bb, hh * NBK:(hh + 1) * NBK]                 # [NBK, P]
        bs_hi = bs.astype(np.float16)
        bs_lo = (bs - bs_hi.astype(np.float64)).astype(np.float16)
        afm_in = np.stack([
            bs_hi.reshape(NBK * P), bs_lo.reshape(NBK * P),
            np.tile(bh, NBK), np.tile(bl, NBK)])
        in_maps.append({
            "xt": xt_in, "w": w_in, "u": u_in,
            "afs": afs_in, "afm": np.ascontiguousarray(afm_in),
            "x12": x12_in,
        })
    return in_maps


def assemble_output(x: np.ndarray, results) -> np.ndarray:
    B, T, Din = x.shape
    out = np.empty((B, T, Din), np.float32)
    out[:, :, ROT:] = x[:, :, ROT:]
    for c in range(N_CORES):
        bb, hh = c // 2, c % 2
        r = results[c]["outT"]                              # [NPAIR*128, 1024]
        blk = (r.reshape(NPAIR, BK, 2, 2, P).transpose(0, 3, 1, 2, 4)
               .reshape(TL, ROT))
        out[bb, hh * TL:(hh + 1) * TL, :ROT] = blk.astype(np.float32)
    return out


def kernel(x: np.ndarray, W: np.ndarray, b: np.ndarray) -> np.ndarray:
    nc = _get_nc()
    in_maps = make_in_maps(x, W, b)
    res = run_bass_kernel_spmd(nc, in_maps, list(range(N_CORES)))
    return assemble_output(x, res.results)
